# revision 14
# baseline (speedup 1.0000x reference)
"""Trainium2 Bass kernel for nn_Dense: out = (x + bias) @ expm(matrix).T, loss = -tr(matrix)/N.

Strategy
--------
Host: compute E = expm(matrix) in float64 (Pade-13 scaling & squaring), fold the
bias through E (bias2 = bias @ E.T), and shard x row-wise across 8 NeuronCores.
Each shard is transposed on the host so the device sees x.T with the
contraction dim (k) on SBUF partitions - no on-device transposes needed.

Device (per core): xt [512, 16384] f32. For each block of TB batch columns:
  DMA xt[:, b0:b0+TB] -> SBUF [128, 4, TB]   (k%128 on partitions, 4 k-chunks)
  for each 128-batch sub-tile: 4 accumulating fp32r matmuls
      psum[128b, 512n] += xt_chunk[k,b].T @ et_chunk[k,n]     (et = E.T)
  DVE adds bias2 (replicated [128,512]) while copying PSUM -> SBUF
  DMA out tile -> out[b0:b0+TB, :]
fp32r is 4-byte fp32 data with the PE's fast (reduced-precision multiply)
path: 1 cycle/row at N=512 vs 4 cycles/row for strict fp32.

The wait-split post-pass works around this container's walrus rejecting
instructions that carry more than one semaphore wait ("Too many sync wait
commands" on the Tile tail drain): excess waits are hoisted onto chained
NoOps on the same engine, which is equivalent (waits execute in program
order ahead of the instruction).
"""

import numpy as np
import bass_rust
import concourse.bass as bass
import concourse.tile as tile
from concourse import mybir
from concourse.bass_utils import run_bass_kernel_spmd

B, N = 131072, 512
NCORES = 8
ROWS = B // NCORES            # 16384 batch rows per core
TB = 1024                     # batch columns per DMA block
NBLK = ROWS // TB
KC = N // 128                 # 4 contraction chunks
SUB = TB // 128               # 128-row sub-tiles per block

_f32 = mybir.dt.float32
_f32r = mybir.dt.float32r


def _expm64(a: np.ndarray) -> np.ndarray:
    """Matrix exponential, float64 Pade-13 scaling and squaring (Higham 2005)."""
    a = np.asarray(a, dtype=np.float64)
    n = a.shape[0]
    ident = np.eye(n, dtype=np.float64)
    b = [
        64764752532480000.0, 32382376266240000.0, 7771770303897600.0,
        1187353796428800.0, 129060195264000.0, 10559470521600.0,
        670442572800.0, 33522128640.0, 1323241920.0, 40840800.0,
        960960.0, 16380.0, 182.0, 1.0,
    ]
    theta13 = 5.371920351148152
    nrm = np.linalg.norm(a, 1)
    s = 0
    if nrm > theta13:
        s = int(np.ceil(np.log2(nrm / theta13)))
    a = a / (2.0 ** s)
    a2 = a @ a
    a4 = a2 @ a2
    a6 = a2 @ a4
    u = a @ (a6 @ (b[13] * a6 + b[11] * a4 + b[9] * a2)
             + b[7] * a6 + b[5] * a4 + b[3] * a2 + b[1] * ident)
    v = (a6 @ (b[12] * a6 + b[10] * a4 + b[8] * a2)
         + b[6] * a6 + b[4] * a4 + b[2] * a2 + b[0] * ident)
    e = np.linalg.solve(v - u, v + u)
    for _ in range(s):
        e = e @ e
    return e


def _split_excess_waits(nc, max_waits: int = 1) -> int:
    """Walrus here only accepts one sem-wait per instruction; hoist extras
    onto chained NoOps (same engine, immediately before the instruction)."""
    nsplit = 0
    for bb in nc.m.functions[0].blocks:
        insts = list(bb.instructions)
        out_insts = []
        changed = False
        for ins in insts:
            si = ins.sync_info
            if si is not None and si.on_wait and len(si.on_wait) > max_waits:
                waits = list(si.on_wait)
                extra, keep = waits[:-max_waits], waits[-max_waits:]
                for i in range(0, len(extra), max_waits):
                    chunk = extra[i : i + max_waits]
                    nop = mybir.InstNoOp(
                        name=f"I-waitsplit-{nsplit}", engine=ins.engine
                    )
                    nop.sync_info = bass_rust.SyncInfo(on_wait=chunk, on_update=[])
                    out_insts.append(nop)
                    nsplit += 1
                ins.sync_info = bass_rust.SyncInfo(
                    on_wait=keep, on_update=list(si.on_update or [])
                )
                changed = True
            out_insts.append(ins)
        if changed:
            try:
                bb.instructions = out_insts
            except Exception:
                bb.clear_instructions()
                for i2 in out_insts:
                    bb.add_instruction(i2)
    return nsplit


def _build_bass():
    nc = bass.Bass()
    # xt is host-pre-tiled: xt[blk, k, j, b] = x_shard[blk*TB + b, j*128 + k],
    # so each block's DMA reads one contiguous KC*TB*4B run per partition.
    xt = nc.declare_dram_parameter("xt", [NBLK, 128, KC, TB], _f32r, isOutput=False)
    et = nc.declare_dram_parameter("et", [N, N], _f32r, isOutput=False)
    biasr = nc.declare_dram_parameter("biasr", [128, N], _f32, isOutput=False)
    out = nc.declare_dram_parameter("out", [ROWS, N], _f32, isOutput=True)
    # partition p holds SUB consecutive output rows -> one contiguous
    # 2KB*SUB descriptor per partition per store-DMA
    out_v = out.rearrange("(blk p bb) n -> blk p bb n", p=128, bb=SUB)

    with tile.TileContext(nc) as tc:
        with (
            tc.tile_pool(name="const", bufs=1) as cpool,
            tc.tile_pool(name="xin", bufs=4) as xpool,
            tc.tile_pool(name="oout", bufs=4) as opool,
            tc.tile_pool(name="psum", bufs=8, space="PSUM") as ppool,
        ):
            et_sb = cpool.tile([128, KC, N], _f32r)
            nc.scalar.dma_start(et_sb[:], et.rearrange("(j k) n -> k j n", k=128))
            biasr_sb = cpool.tile([128, N], _f32)
            nc.scalar.dma_start(biasr_sb[:], biasr[:])

            for blk in range(NBLK):
                xt_t = xpool.tile([128, KC, TB], _f32r)
                nc.sync.dma_start(xt_t[:], xt[blk])
                out_t = opool.tile([128, SUB, N], _f32)
                for bb in range(SUB):
                    ps = ppool.tile([128, N], _f32)
                    for j in range(KC):
                        # batch column for out partition p is p*SUB + bb
                        nc.tensor.matmul(
                            ps[:],
                            xt_t[:, j, bb : bb + (128 - 1) * SUB + 1 : SUB],
                            et_sb[:, j, :],
                            start=(j == 0),
                            stop=(j == KC - 1),
                        )
                    nc.vector.tensor_add(out_t[:, bb, :], ps[:], biasr_sb[:])
                nc.scalar.dma_start(out_v[blk], out_t[:])

    _split_excess_waits(nc)
    return nc


_NC_CACHE = None


def _get_nc():
    global _NC_CACHE
    if _NC_CACHE is None:
        _NC_CACHE = _build_bass()
    return _NC_CACHE


def kernel(x, matrix, bias):
    x = np.asarray(x, dtype=np.float32)
    matrix = np.asarray(matrix, dtype=np.float32)
    bias = np.asarray(bias, dtype=np.float32)

    e64 = _expm64(matrix)
    et = np.ascontiguousarray(e64.T).astype(np.float32)             # et[k, n] = E[n, k]
    bias2 = (np.asarray(bias, np.float64) @ e64.T).astype(np.float32)
    biasr = np.ascontiguousarray(
        np.broadcast_to(bias2, (128, N))
    ).astype(np.float32)

    in_maps = []
    for c in range(NCORES):
        xs = x[c * ROWS : (c + 1) * ROWS]                           # [16384, 512]
        # xt[blk, k, j, b] = xs[blk*TB + b, j*128 + k]
        xt = np.ascontiguousarray(
            xs.reshape(NBLK, TB, KC, 128).transpose(0, 3, 2, 1)
        )
        in_maps.append({"xt": xt, "et": et, "biasr": biasr})

    res = run_bass_kernel_spmd(_get_nc(), in_maps, list(range(NCORES)))
    out = np.concatenate(
        [res.results[c]["out"] for c in range(NCORES)], axis=0
    )
    loss = np.float32(-np.trace(matrix.astype(np.float64)) / N)
    return out, loss


# revision 15
# speedup vs baseline: 1.0765x; 1.0765x over previous
"""Trainium2 Bass kernel for nn_Dense: out = (x + bias) @ expm(matrix).T, loss = -tr(matrix)/N.

Strategy
--------
Host: compute E = expm(matrix) in float64 (Pade-13 scaling & squaring), fold the
bias through E (bias2 = bias @ E.T), and shard x row-wise across 8 NeuronCores.
Each shard is transposed on the host so the device sees x.T with the
contraction dim (k) on SBUF partitions - no on-device transposes needed.

Device (per core): xt [512, 16384] f32. For each block of TB batch columns:
  DMA xt[:, b0:b0+TB] -> SBUF [128, 4, TB]   (k%128 on partitions, 4 k-chunks)
  for each 128-batch sub-tile: 4 accumulating fp32r matmuls
      psum[128b, 512n] += xt_chunk[k,b].T @ et_chunk[k,n]     (et = E.T)
  DVE adds bias2 (replicated [128,512]) while copying PSUM -> SBUF
  DMA out tile -> out[b0:b0+TB, :]
fp32r is 4-byte fp32 data with the PE's fast (reduced-precision multiply)
path: 1 cycle/row at N=512 vs 4 cycles/row for strict fp32.

The wait-split post-pass works around this container's walrus rejecting
instructions that carry more than one semaphore wait ("Too many sync wait
commands" on the Tile tail drain): excess waits are hoisted onto chained
NoOps on the same engine, which is equivalent (waits execute in program
order ahead of the instruction).
"""

import numpy as np
import bass_rust
import concourse.bass as bass
import concourse.tile as tile
from concourse import mybir
from concourse.bass_utils import run_bass_kernel_spmd

B, N = 131072, 512
NCORES = 8
ROWS = B // NCORES            # 16384 batch rows per core
TB = 1024                     # batch columns per DMA block
NBLK = ROWS // TB
KC = N // 128                 # 4 contraction chunks
SUB = TB // 128               # 128-row sub-tiles per block

_f32 = mybir.dt.float32
_f32r = mybir.dt.float32r


def _expm64(a: np.ndarray) -> np.ndarray:
    """Matrix exponential, float64 Pade-13 scaling and squaring (Higham 2005)."""
    a = np.asarray(a, dtype=np.float64)
    n = a.shape[0]
    ident = np.eye(n, dtype=np.float64)
    b = [
        64764752532480000.0, 32382376266240000.0, 7771770303897600.0,
        1187353796428800.0, 129060195264000.0, 10559470521600.0,
        670442572800.0, 33522128640.0, 1323241920.0, 40840800.0,
        960960.0, 16380.0, 182.0, 1.0,
    ]
    theta13 = 5.371920351148152
    nrm = np.linalg.norm(a, 1)
    s = 0
    if nrm > theta13:
        s = int(np.ceil(np.log2(nrm / theta13)))
    a = a / (2.0 ** s)
    a2 = a @ a
    a4 = a2 @ a2
    a6 = a2 @ a4
    u = a @ (a6 @ (b[13] * a6 + b[11] * a4 + b[9] * a2)
             + b[7] * a6 + b[5] * a4 + b[3] * a2 + b[1] * ident)
    v = (a6 @ (b[12] * a6 + b[10] * a4 + b[8] * a2)
         + b[6] * a6 + b[4] * a4 + b[2] * a2 + b[0] * ident)
    e = np.linalg.solve(v - u, v + u)
    for _ in range(s):
        e = e @ e
    return e


def _split_excess_waits(nc, max_waits: int = 1) -> int:
    """Walrus here only accepts one sem-wait per instruction; hoist extras
    onto chained NoOps (same engine, immediately before the instruction)."""
    nsplit = 0
    for bb in nc.m.functions[0].blocks:
        insts = list(bb.instructions)
        out_insts = []
        changed = False
        for ins in insts:
            si = ins.sync_info
            if si is not None and si.on_wait and len(si.on_wait) > max_waits:
                waits = list(si.on_wait)
                extra, keep = waits[:-max_waits], waits[-max_waits:]
                for i in range(0, len(extra), max_waits):
                    chunk = extra[i : i + max_waits]
                    nop = mybir.InstNoOp(
                        name=f"I-waitsplit-{nsplit}", engine=ins.engine
                    )
                    nop.sync_info = bass_rust.SyncInfo(on_wait=chunk, on_update=[])
                    out_insts.append(nop)
                    nsplit += 1
                ins.sync_info = bass_rust.SyncInfo(
                    on_wait=keep, on_update=list(si.on_update or [])
                )
                changed = True
            out_insts.append(ins)
        if changed:
            try:
                bb.instructions = out_insts
            except Exception:
                bb.clear_instructions()
                for i2 in out_insts:
                    bb.add_instruction(i2)
    return nsplit


def _build_bass():
    nc = bass.Bass()
    # xt is host-pre-tiled: xt[blk, k, j, b] = x_shard[blk*TB + b, j*128 + k],
    # so each block's DMA reads one contiguous KC*TB*4B run per partition.
    xt = nc.declare_dram_parameter("xt", [NBLK, 128, KC, TB], _f32r, isOutput=False)
    et = nc.declare_dram_parameter("et", [N, N], _f32r, isOutput=False)
    biasr = nc.declare_dram_parameter("biasr", [128, N], _f32, isOutput=False)
    out = nc.declare_dram_parameter("out", [ROWS, N], _f32, isOutput=True)
    # partition p holds SUB consecutive output rows -> one contiguous
    # 2KB*SUB descriptor per partition per store-DMA
    out_v = out.rearrange("(blk p bb) n -> blk p bb n", p=128, bb=SUB)

    with tile.TileContext(nc) as tc:
        with (
            tc.tile_pool(name="const", bufs=1) as cpool,
            tc.tile_pool(name="xin", bufs=4) as xpool,
            tc.tile_pool(name="oout", bufs=4) as opool,
            tc.tile_pool(name="psum", bufs=8, space="PSUM") as ppool,
        ):
            # constants go FIRST on the same HWDGE queue as the xt stream:
            # per-queue FIFO guarantees they land before the xt flood, so the
            # PE can start ~10us in (a separate queue gets round-robin
            # starved behind 32MB of xt packets -> 20us idle head).
            et_sb = cpool.tile([128, KC, N], _f32r)
            nc.sync.dma_start(et_sb[:], et.rearrange("(j k) n -> k j n", k=128))
            biasr_sb = cpool.tile([128, N], _f32)
            nc.sync.dma_start(biasr_sb[:], biasr[:])

            for blk in range(NBLK):
                xt_t = xpool.tile([128, KC, TB], _f32r)
                nc.sync.dma_start(xt_t[:], xt[blk])
                out_t = opool.tile([128, SUB, N], _f32)
                for bb in range(SUB):
                    ps = ppool.tile([128, N], _f32)
                    for j in range(KC):
                        # batch column for out partition p is p*SUB + bb
                        nc.tensor.matmul(
                            ps[:],
                            xt_t[:, j, bb : bb + (128 - 1) * SUB + 1 : SUB],
                            et_sb[:, j, :],
                            start=(j == 0),
                            stop=(j == KC - 1),
                        )
                    nc.vector.tensor_add(out_t[:, bb, :], ps[:], biasr_sb[:])
                nc.scalar.dma_start(out_v[blk], out_t[:])

    _split_excess_waits(nc)
    return nc


_NC_CACHE = None


def _get_nc():
    global _NC_CACHE
    if _NC_CACHE is None:
        _NC_CACHE = _build_bass()
    return _NC_CACHE


def kernel(x, matrix, bias):
    x = np.asarray(x, dtype=np.float32)
    matrix = np.asarray(matrix, dtype=np.float32)
    bias = np.asarray(bias, dtype=np.float32)

    e64 = _expm64(matrix)
    et = np.ascontiguousarray(e64.T).astype(np.float32)             # et[k, n] = E[n, k]
    bias2 = (np.asarray(bias, np.float64) @ e64.T).astype(np.float32)
    biasr = np.ascontiguousarray(
        np.broadcast_to(bias2, (128, N))
    ).astype(np.float32)

    in_maps = []
    for c in range(NCORES):
        xs = x[c * ROWS : (c + 1) * ROWS]                           # [16384, 512]
        # xt[blk, k, j, b] = xs[blk*TB + b, j*128 + k]
        xt = np.ascontiguousarray(
            xs.reshape(NBLK, TB, KC, 128).transpose(0, 3, 2, 1)
        )
        in_maps.append({"xt": xt, "et": et, "biasr": biasr})

    res = run_bass_kernel_spmd(_get_nc(), in_maps, list(range(NCORES)))
    out = np.concatenate(
        [res.results[c]["out"] for c in range(NCORES)], axis=0
    )
    loss = np.float32(-np.trace(matrix.astype(np.float64)) / N)
    return out, loss
